# revision 42
# baseline (speedup 1.0000x reference)
"""Distributed multi-head attention kernel for 8 TRN2 NeuronCores.

Sharding: 8-way head parallel (2 heads per core), batches looped on-core.
Each core: QKV projection for its 2 heads over both batches, per-head
attention (softmax without max-subtraction — logits are small; denominators
come from a ones-column appended to V so they fall out of the attn@V
matmul), then per-head AllToAlls across all 8 cores exchange row-blocks
for head-blocks: block s = (batch s//4, rows-block s%4).  Core c ends up
with all 1024 inner dims for (batch c//4, rows [(c%4)*512, ...)) and runs
the full output projection + bias on that slice.

The attention middle is ACT-bound: 16.8M exps per core = ~145us of
Scalar-engine streaming, vs ~4.3us of PE work per 512-row block.  The
whole schedule is therefore built around keeping the exp stream dense:

- q/k are cast to fp8e4 post-projection so the QK^T matmuls run in
  DoubleRow perf mode (half the stream cycles; the DoubleRow k-tile
  pair dim is a stride-0 broadcast, compensated by halving the exp
  scale).  Costs ~0.9% relative error (1.3e-2 total, budget 2e-2),
  frees PE slack that the filler schedule spends.
- The first exp fires ~25us in: x^T row-block 0 fans out over 3 DMA
  queues, and the first ST block's jg groups interleave with the
  remaining k-projection chunks (group j only needs k block j//2).
- Every other projection (q/k rest, v of both batches) is chopped into
  ~1us filler units emitted between ST groups, sized so no unit delays
  the next ACT; q/k chunks split into two 4-ko halves with the PSUM
  accumulation held open across the gap.
- Phases run batch-major (h0b0, h1b0, h0b1, h1b1) so batch-1
  projections spread over two phases of filler slots; the head-0 A2A
  still hides under the last phase.
- A dummy AllGather at kernel start absorbs core boot skew so the real
  A2As (which share one serialized CC stream) fly at data-limited time.
- The head-1 receive interleaves the output projection: as core i's
  columns land in attnT, contraction step ko=i runs for output chunks
  0-5 (3 spare PSUM tiles = 6 open accumulations), overlapping most of
  the final matmuls with the receive chain.  Bias-adds ride the idle
  Scalar engine.  Junk "warm" matmuls cover the A2A-1 flight (HAM
  down-throttles the PE to 4/8 issue after idle gaps).

x and the weights are cast to bf16 on the host (bf16 is the compute
precision anyway) and x additionally arrives pre-transposed, so x^T
streams straight into SBUF with fully contiguous DMAs.  A2A-receive
transposes run on the PE array (XBAR transposes proved slow for narrow
blocks and corrupt data when issued on two HWDGE queues concurrently).

The per-core output is the TRANSPOSED final slice [1024, 512] (PSUM-major
writes stay contiguous); the host transposes during assembly.
"""
import numpy as np

import concourse.bass as bass
import concourse.mybir as mybir
from concourse import bacc
import concourse.tile as tile
from concourse.bass_utils import run_bass_kernel_spmd
from concourse.masks import make_identity

# problem constants (hardcoded; kernel.py must be self-contained)
B, N, DIM = 2, 2048, 1024
H, DH = 16, 64
INNER = H * DH            # 1024
SCALE = DIM ** -0.5       # 1/32  (module scales by dim**-0.5, not dim_head)
NCORES = 8
HPC = H // NCORES         # 2 heads per core
SH = HPC * DH             # 128 inner cols per core
ROWS = N // 4             # 512 output rows per core
P = 128
KO = DIM // P             # 8 contraction chunks
JC = N // P               # 16 row chunks
IB = 512                  # query block size
NIB = N // IB             # 4 query blocks
ISUB = IB // P            # 4
FP32 = mybir.dt.float32
BF16 = mybir.dt.bfloat16
F8 = mybir.dt.float8e4

REPLICA_GROUPS = [[0, 1, 2, 3, 4, 5, 6, 7]]

_NC_CACHE = {}

# set by the last kernel() call when BASS_KERNEL_TRACE=1 (for test.py)
LAST_RESULTS = None


def _build():
    nc = bacc.Bacc(num_devices=NCORES)

    x_ext = nc.declare_dram_parameter("x", [B * DIM, N], BF16, isOutput=False)
    wq_ext = nc.declare_dram_parameter("wq", [DIM, SH], BF16, isOutput=False)
    wk_ext = nc.declare_dram_parameter("wk", [DIM, SH], BF16, isOutput=False)
    wv_ext = nc.declare_dram_parameter("wv", [DIM, SH], BF16, isOutput=False)
    wo_ext = nc.declare_dram_parameter("wo", [DIM, DIM], BF16, isOutput=False)
    bo_ext = nc.declare_dram_parameter("bo", [DIM], FP32, isOutput=False)
    out_ext = nc.declare_dram_parameter("out", [DIM, ROWS], FP32, isOutput=True)

    with tile.TileContext(nc) as tc:
        with (
            tc.tile_pool(name="consts", bufs=1) as consts,
            tc.tile_pool(name="stage", bufs=3) as stage,
            tc.tile_pool(name="xt_pool", bufs=2) as xt_pool,
            tc.tile_pool(name="pt_pool", bufs=4) as pt_pool,
            tc.tile_pool(name="nrm", bufs=8) as nrm,
            tc.tile_pool(name="rst_pool", bufs=8) as rst_pool,
            tc.tile_pool(name="dram", bufs=1, space="DRAM") as dram,
            tc.tile_pool(name="st_psum", bufs=3, space="PSUM") as st_psum,
            tc.tile_pool(name="o_psum", bufs=2, space="PSUM") as o_psum,
        ):
            ident_bf = consts.tile([P, P], BF16)

            # persistent tensors
            wq_sb = consts.tile([P, KO, SH], BF16)
            wk_sb = consts.tile([P, KO, SH], BF16)
            wv_sb = consts.tile([P, KO, SH], BF16)
            wo_sb = consts.tile([P, KO, DIM], BF16)
            bias_sb = consts.tile([P, KO], FP32)
            # q/k kept in fp8e4 so the ST matmuls can use DoubleRow perf
            # mode (2 rows/cycle).  The DoubleRow "2 k-tiles" dim is a
            # stride-0 broadcast (both slots read the same data), which
            # doubles the logits; the exp ACT scale is halved to compensate.
            qT = consts.tile([P, B, HPC, N], F8)     # [d(+zero pad), b, h, i]
            kT = consts.tile([P, B, HPC, N], F8)
            v_aug = consts.tile([P, B, JC, HPC, DH + 1], BF16)
            out_rows = consts.tile([P, B, JC, SH], BF16)
            attnT = xt_pool.tile([P, KO, N], BF16, tag="xT", name="attnT")[
                :, :, :ROWS
            ]

            # tiny sync collective fired at kernel start: absorbs core
            # boot skew during the projection ramp so the real A2As fly
            # at their data-limited time instead of inheriting the skew
            # (the CC stream serializes collectives, so a skew-inflated
            # A2A-0 otherwise delays A2A-1's trigger).
            sync_in = dram.tile([P], BF16, name="sync_in")
            sync_out = dram.tile([NCORES, P], BF16, name="sync_out")
            a2a_in0 = dram.tile([NCORES, P, NIB, DH], BF16, name="a2a_in0")
            a2a_in1 = dram.tile([NCORES, P, NIB, DH], BF16, name="a2a_in1")
            a2a_out0 = dram.tile([NCORES, P, NIB, DH], BF16, name="a2a_out0")
            a2a_out1 = dram.tile([NCORES, P, NIB, DH], BF16, name="a2a_out1")
            a2a_ins = [a2a_in0, a2a_in1]
            a2a_outs = [a2a_out0, a2a_out1]

            # batch-0 q/k contraction pads zeroed on the Scalar engine
            # (idle until the first exp; keeps DVE free for the first q/k
            # casts and the gpsimd queue free for weight/xT DMAs);
            # batch-1 pads are emitted later, off the critical path.
            nc.scalar.memzero(qT[DH:P, 0, :, :])
            nc.scalar.memzero(kT[DH:P, 0, :, :])
            nc.vector.memset(v_aug[:, :, :, :, DH : DH + 1], 1.0)
            make_identity(nc, ident_bf)

            # junk operand for PE warm-up / warm-keeper matmuls (HAM needs
            # ~3.4us of sustained PE activity to un-throttle 1.2->2.4 GHz)
            junk = consts.tile([P, IB], BF16)
            nc.vector.memset(junk, 0.125)

            def pe_warm(n):
                for _ in range(n):
                    psw = st_psum.tile([P, 2, IB], FP32, tag="st", name="warm")
                    nc.tensor.matmul(
                        psw[:, 0, :], junk[:, 0:P], junk, start=True, stop=True
                    )

            def load_xT(b):
                """x[b]^T arrives pre-transposed from the host: straight
                contiguous loads spread over the DMA queues.  For batch 0
                the first row-block fans out over THREE queues (scalar is
                still idle pre-exp) so the k projection unblocks ~5us in."""
                xT = xt_pool.tile([P, KO, N], BF16, tag="xT", name="xT")
                for nb in range(NIB):
                    for ko in range(KO):
                        if b == 0 and nb == 0:
                            eng = [nc.sync, nc.gpsimd, nc.scalar][ko % 3]
                        else:
                            eng = [nc.sync, nc.gpsimd][ko % 2]
                        eng.dma_start(
                            xT[:, ko, nb * IB : (nb + 1) * IB],
                            x_ext[
                                b * DIM + ko * P : b * DIM + (ko + 1) * P,
                                nb * IB : (nb + 1) * IB,
                            ],
                        )
                return xT

            qk_open = {}

            def qk_half(b, xT, w_sb, dstT, nb, half):
                """One half (4 ko steps) of a q/k projection chunk; the
                PSUM accumulation stays open between halves so a half fits
                a single filler slot without overrunning the ACT cadence."""
                key = (id(dstT), b, nb)
                if half == 0:
                    ps2 = st_psum.tile([P, 2, IB], FP32, tag="st", name="qk_ps")
                    qk_open[key] = ps2
                ps = qk_open[key][:, 0, :]
                for ko in range(half * 4, half * 4 + 4):
                    nc.tensor.matmul(
                        ps,
                        w_sb[:, ko, :],
                        xT[:, ko, nb * IB : (nb + 1) * IB],
                        start=(ko == 0),
                        stop=(ko == KO - 1),
                    )
                if half == 1:
                    del qk_open[key]
                    for h in range(HPC):
                        nc.vector.tensor_copy(
                            dstT[0:DH, b, h, nb * IB : (nb + 1) * IB],
                            ps[h * DH : (h + 1) * DH, :],
                        )

            def qk_chunk(b, xT, w_sb, dstT, nbs):
                for nb in nbs:
                    qk_half(b, xT, w_sb, dstT, nb, 0)
                    qk_half(b, xT, w_sb, dstT, nb, 1)

            def qk_proj(b, xT):
                # k first (attention's dots consume kT earliest), then q
                qk_chunk(b, xT, wk_sb, kT, range(NIB))
                qk_chunk(b, xT, wq_sb, qT, range(NIB))

            def load_qkv_weights():
                for w_ext, w_sb in (
                    (wk_ext, wk_sb),
                    (wq_ext, wq_sb),
                    (wv_ext, wv_sb),
                ):
                    nc.gpsimd.dma_start(
                        w_sb, w_ext.rearrange("(ko kp) c -> kp ko c", kp=P)
                    )

            def load_out_weights():
                nc.gpsimd.dma_start(
                    wo_sb, wo_ext.rearrange("(ko kp) c -> kp ko c", kp=P)
                )
                nc.gpsimd.dma_start(
                    bias_sb, bo_ext.rearrange("(co cp) -> cp co", cp=P)
                )

            def v_proj(b, xT, mts=None):
                for mt in (range(JC) if mts is None else mts):
                    psv2 = st_psum.tile([P, 2, IB], FP32, tag="st", name="v_ps")
                    ps_v = psv2[:, 0, :SH]
                    for ko in range(KO):
                        nc.tensor.matmul(
                            ps_v,
                            xT[:, ko, mt * P : (mt + 1) * P],
                            wv_sb[:, ko, :],
                            start=(ko == 0),
                            stop=(ko == KO - 1),
                        )
                    nc.vector.tensor_copy(
                        v_aug[:, b, mt, :, 0:DH],
                        ps_v.rearrange("p (h d) -> p h d", d=DH),
                    )

            ptiles = {}

            def st_exp(h, b, ib, fillers=None):
                """ST = k@q.T per j-chunk (fp8 DoubleRow), exp on ACT.
                fillers: up to 8 closures of ~1-2us of PE work, one
                emitted before each jg group, so projection work streams
                through ACT-bound rounds without ever parking more than
                one filler unit ahead of the next ST group."""
                ptile = pt_pool.tile([P, JC, IB], BF16, tag="pt", name="ptile")
                ptiles[(h, b, ib)] = ptile
                for jg in range(JC // 2):
                    if fillers is not None and jg < len(fillers):
                        f = fillers[jg]
                        if f is not None:
                            f()
                    ps_st = st_psum.tile(
                        [P, 2, IB], FP32, tag="st", name="st_ps"
                    )
                    for u in range(2):
                        jc = jg * 2 + u
                        nc.tensor.matmul(
                            ps_st[:, u, :],
                            kT[:, None, b, h, jc * P : (jc + 1) * P]
                            .broadcast_to([P, 2, P]),
                            qT[:, None, b, h, ib * IB : (ib + 1) * IB]
                            .broadcast_to([P, 2, IB]),
                            start=True,
                            stop=True,
                            perf_mode=mybir.MatmulPerfMode.DoubleRow,
                        )
                    nc.scalar.activation(
                        ptile[:, jg * 2 : (jg + 1) * 2, :],
                        ps_st,
                        mybir.ActivationFunctionType.Exp,
                        scale=SCALE / 2,
                    )

            def av(h, b, ib, weave=False):
                """attn@V with the denominator in column DH.  All 4 i-sub
                accumulations of a block share one PSUM bank so the 'po'
                slots rotate once per block (normalizes emitted after all
                64 matmuls so the bank tracker doesn't interleave PE
                writes with DVE reads)."""
                po = h * DH
                ptile = ptiles.pop((h, b, ib))
                ps_o4 = o_psum.tile(
                    [P, ISUB, DH + 1], FP32, tag="po", name="o_ps"
                )
                for isub in range(ISUB):
                    if weave:
                        pe_warm(1)
                    for jc in range(JC):
                        nc.tensor.matmul(
                            ps_o4[:, isub, :],
                            ptile[:, jc, isub * P : (isub + 1) * P],
                            v_aug[:, b, jc, h, :],
                            start=(jc == 0),
                            stop=(jc == JC - 1),
                        )
                for isub in range(ISUB):
                    ic = ib * ISUB + isub
                    recip = nrm.tile([P, 1], FP32, tag="recip", name="recip")
                    nc.vector.reciprocal(recip, ps_o4[:, isub, DH : DH + 1])
                    nc.vector.tensor_scalar_mul(
                        out_rows[:, b, ic, po : po + DH],
                        ps_o4[:, isub, 0:DH],
                        recip,
                    )
                # block (h, b, ib) complete -> stage its A2A input
                s = b * NIB + ib
                nc.sync.dma_start(
                    a2a_ins[h][s],
                    out_rows[
                        :, b, ib * ISUB : (ib + 1) * ISUB, po : po + DH
                    ],
                )

            def a2a_exchange(h):
                nc.gpsimd.collective_compute(
                    "AllToAll",
                    mybir.AluOpType.bypass,
                    replica_groups=REPLICA_GROUPS,
                    ins=[a2a_ins[h].opt()],
                    outs=[a2a_outs[h].opt()],
                )

            def receive(h, eng=None):
                po = h * DH
                eng = eng or nc.sync
                rstages = []
                for i in range(NCORES):
                    rstage = rst_pool.tile(
                        [P, NIB, DH], BF16, tag="rstage", name="rstage"
                    )
                    eng.dma_start(rstage, a2a_outs[h][i])
                    rstages.append(rstage)
                for i in range(NCORES):
                    rps = st_psum.tile([DH, NIB, P], BF16, tag="st", name="r_ps")
                    for q in range(NIB):
                        nc.tensor.transpose(
                            rps[:, q, :], rstages[i][:, q, :], ident_bf
                        )
                    nc.vector.tensor_copy(attnT[po : po + DH, i, :], rps)

            def emit_out(ps_f, cc):
                # bias-add on the Scalar engine (idle after the last exp;
                # keeps DVE free for the receive copies in the tail)
                of = stage.tile([P, ROWS], FP32, tag="of", name="of")
                nc.scalar.activation(
                    of,
                    ps_f,
                    mybir.ActivationFunctionType.Identity,
                    bias=bias_sb[:, cc : cc + 1],
                )
                nc.sync.dma_start(out_ext[cc * P : (cc + 1) * P, :], of)

            def receive1_final():
                """Head-1 receive with the output projection interleaved:
                as each source core's columns land in attnT, the matching
                ko contraction step runs for output chunks 0-5 (3 spare
                PSUM st tiles = 6 open accumulations; chunks 6-7 run
                monolithically at the end).  Overlaps ~13us of final
                matmuls with the receive chain instead of serializing."""
                po = DH
                rstages = []
                for i in range(NCORES):
                    rstage = rst_pool.tile(
                        [P, NIB, DH], BF16, tag="rstage", name="rstage"
                    )
                    (nc.sync if i % 2 == 0 else nc.scalar).dma_start(
                        rstage, a2a_outs[1][i]
                    )
                    rstages.append(rstage)
                fps = [
                    st_psum.tile([P, 2, IB], FP32, tag="st", name="f_ps")
                    for _ in range(3)
                ]
                for i in range(NCORES):
                    rps = o_psum.tile([DH, NIB, P], BF16, tag="po", name="r_ps")
                    for q in range(NIB):
                        nc.tensor.transpose(
                            rps[:, q, :], rstages[i][:, q, :], ident_bf
                        )
                    nc.vector.tensor_copy(attnT[po : po + DH, i, :], rps)
                    for cc in range(6):
                        nc.tensor.matmul(
                            fps[cc // 2][:, cc % 2, :ROWS],
                            wo_sb[:, i, cc * P : (cc + 1) * P],
                            attnT[:, i, :],
                            start=(i == 0),
                            stop=(i == KO - 1),
                        )
                for cc in range(6):
                    emit_out(fps[cc // 2][:, cc % 2, :ROWS], cc)
                for cc in range(6, KO):
                    psf2 = st_psum.tile([P, 2, IB], FP32, tag="st", name="f_ps")
                    ps_f = psf2[:, 0, :ROWS]
                    for ko in range(KO):
                        nc.tensor.matmul(
                            ps_f,
                            wo_sb[:, ko, cc * P : (cc + 1) * P],
                            attnT[:, ko, :],
                            start=(ko == 0),
                            stop=(ko == KO - 1),
                        )
                    emit_out(ps_f, cc)

            # ---- program order: ACT-dense schedule.  The attention phase
            # is ACT-bound (~9us of exp per block vs ~4.3us of PE work with
            # fp8 STs), so the first exp is pulled as early as possible
            # (k(0) + q(0) block 0 + ST block 0 only) and every remaining
            # projection is emitted as PE filler between ST groups, sized
            # so each ST group lands before the ACT queue drains.  Phases
            # run batch-major (h0b0, h1b0, h0b1, h1b1) so batch-1
            # projections can spread across TWO phases of filler; the
            # head-0 A2A still fully hides under phase h1b1. ----
            kh = lambda b, xT, nb, hf: (
                lambda: qk_half(b, xT, wk_sb, kT, nb, hf)
            )
            qh = lambda b, xT, nb, hf: (
                lambda: qk_half(b, xT, wq_sb, qT, nb, hf)
            )
            qc = lambda b, xT, nb: (lambda: qk_chunk(b, xT, wq_sb, qT, [nb]))
            vc = lambda b, xT, m0, m1: (lambda: v_proj(b, xT, range(m0, m1)))
            wu = lambda: pe_warm(1)

            load_qkv_weights()
            xT0 = load_xT(0)
            nc.gpsimd.collective_compute(
                "AllGather",
                mybir.AluOpType.bypass,
                replica_groups=REPLICA_GROUPS,
                ins=[sync_in.opt()],
                outs=[sync_out.opt()],
            )
            pe_warm(2)
            qk_chunk(0, xT0, wk_sb, kT, [0])
            qk_chunk(0, xT0, wq_sb, qT, [0])
            # first ST block interleaves with the remaining k chunks (its
            # jg group j only needs the k block j//2 complete), so exp
            # starts right after the first k/q blocks land.
            st_exp(0, 0, 0, fillers=[
                kh(0, xT0, 1, 0), kh(0, xT0, 1, 1),
                kh(0, xT0, 2, 0), kh(0, xT0, 2, 1),
                kh(0, xT0, 3, 0), kh(0, xT0, 3, 1),
                qh(0, xT0, 1, 0), qh(0, xT0, 1, 1),
            ])
            # batch-1 q/k pads, off the startup critical path
            nc.gpsimd.memset(qT[DH:P, 1, :, :], 0.0)
            nc.gpsimd.memset(kT[DH:P, 1, :, :], 0.0)
            st_exp(0, 0, 1, fillers=[
                qh(0, xT0, 2, 0), qh(0, xT0, 2, 1),
                vc(0, xT0, 0, 1), vc(0, xT0, 1, 2), vc(0, xT0, 2, 3),
                vc(0, xT0, 3, 4), vc(0, xT0, 4, 5), vc(0, xT0, 5, 6),
            ])
            xT1 = load_xT(1)
            load_out_weights()
            st_exp(0, 0, 2, fillers=[
                qh(0, xT0, 3, 0), qh(0, xT0, 3, 1),
                vc(0, xT0, 6, 7), vc(0, xT0, 7, 8), vc(0, xT0, 8, 9),
                vc(0, xT0, 9, 10), vc(0, xT0, 10, 11), vc(0, xT0, 11, 12),
            ])
            st_exp(0, 0, 3, fillers=[
                vc(0, xT0, 12, 13), vc(0, xT0, 13, 14), vc(0, xT0, 14, 15),
                vc(0, xT0, 15, 16), vc(1, xT1, 0, 1), vc(1, xT1, 1, 2),
                vc(1, xT1, 2, 3), vc(1, xT1, 3, 4),
            ])
            # phase h1b0: fillers = batch-1 k chunks (halves in adjacent
            # slots so the open PSUM chain spans only one ST group) + v
            av(0, 0, 0)
            st_exp(1, 0, 0, fillers=[
                kh(1, xT1, 0, 0), kh(1, xT1, 0, 1),
                vc(1, xT1, 4, 5), None, vc(1, xT1, 5, 6), None,
            ])
            av(0, 0, 1)
            st_exp(1, 0, 1, fillers=[
                kh(1, xT1, 1, 0), kh(1, xT1, 1, 1),
                vc(1, xT1, 6, 7), None, vc(1, xT1, 7, 8), None,
            ])
            av(0, 0, 2)
            st_exp(1, 0, 2, fillers=[
                kh(1, xT1, 2, 0), kh(1, xT1, 2, 1),
                vc(1, xT1, 8, 9), None, vc(1, xT1, 9, 10), None,
            ])
            av(0, 0, 3)
            st_exp(1, 0, 3, fillers=[
                kh(1, xT1, 3, 0), kh(1, xT1, 3, 1),
                vc(1, xT1, 10, 11), None, vc(1, xT1, 11, 12), None,
            ])
            # phase h0b1: fillers = batch-1 q chunks, just in time (the
            # slot-0 q chunk feeds this very block's STs)
            av(1, 0, 0)
            st_exp(0, 1, 0, fillers=[
                qc(1, xT1, 0), vc(1, xT1, 12, 13), None,
                vc(1, xT1, 13, 14), None, None,
            ])
            av(1, 0, 1)
            st_exp(0, 1, 1, fillers=[
                qc(1, xT1, 1), vc(1, xT1, 14, 15), None,
                vc(1, xT1, 15, 16), None, None,
            ])
            av(1, 0, 2)
            st_exp(0, 1, 2, fillers=[
                qc(1, xT1, 2), None, None, None, None
            ])
            av(1, 0, 3)
            st_exp(0, 1, 3, fillers=[
                qc(1, xT1, 3), None, None, None, None
            ])
            # phase h1b1; head-0 finishes at av(0,1,3) -> exchange(0)
            # flies under the tail of this phase.  Small warm fillers keep
            # the PE above HAM's low-activity threshold now that all real
            # projection work is done.
            av(0, 1, 0)
            st_exp(1, 1, 0, fillers=[wu, None, None, None, wu])
            av(0, 1, 1)
            st_exp(1, 1, 1, fillers=[wu, None, None, None, wu])
            av(0, 1, 2)
            st_exp(1, 1, 2, fillers=[wu, None, None, None, wu])
            av(0, 1, 3)
            a2a_exchange(0)
            st_exp(1, 1, 3, fillers=[wu, None, None, None, wu])
            av(1, 1, 0, weave=True)
            av(1, 1, 1, weave=True)
            av(1, 1, 2, weave=True)
            av(1, 1, 3, weave=True)
            a2a_exchange(1)
            receive(0, nc.scalar)  # coll-0 landed by now; ACT queue free
            pe_warm(40)            # PE activity across the A2A-1 flight
            receive1_final()

    nc.finalize()
    return nc


def _get_nc():
    if "nc" not in _NC_CACHE:
        _NC_CACHE["nc"] = _build()
    return _NC_CACHE["nc"]


def kernel(**inputs) -> np.ndarray:
    import os

    import ml_dtypes

    global LAST_RESULTS

    bf16 = ml_dtypes.bfloat16
    x = np.asarray(inputs["x"], dtype=np.float32)
    W_qkv = np.asarray(inputs["W_qkv"], dtype=np.float32)
    W_out = np.asarray(inputs["W_out"], dtype=np.float32)
    b_out = np.ascontiguousarray(np.asarray(inputs["b_out"], dtype=np.float32))

    x_bf = np.ascontiguousarray(
        x.transpose(0, 2, 1).reshape(B * DIM, N).astype(bf16)
    )
    wo_bf = np.ascontiguousarray(W_out.astype(bf16))
    wqkv_bf = W_qkv.astype(bf16)

    nc = _get_nc()

    in_maps = []
    for c in range(NCORES):
        in_maps.append(
            {
                "x": x_bf,
                "wq": np.ascontiguousarray(
                    wqkv_bf[:, 0 * INNER + c * SH : 0 * INNER + (c + 1) * SH]
                ),
                "wk": np.ascontiguousarray(
                    wqkv_bf[:, 1 * INNER + c * SH : 1 * INNER + (c + 1) * SH]
                ),
                "wv": np.ascontiguousarray(
                    wqkv_bf[:, 2 * INNER + c * SH : 2 * INNER + (c + 1) * SH]
                ),
                "wo": wo_bf,
                "bo": b_out,
            }
        )

    trace = os.environ.get("BASS_KERNEL_TRACE", "0") == "1"
    res = run_bass_kernel_spmd(
        nc, in_maps, core_ids=list(range(NCORES)), trace=trace
    )
    LAST_RESULTS = res

    y = np.empty((B, N, DIM), dtype=np.float32)
    for c in range(NCORES):
        b, r = c // 4, c % 4
        y[b, r * ROWS : (r + 1) * ROWS, :] = res.results[c]["out"].T
    return y



# revision 44
# speedup vs baseline: 1.0263x; 1.0263x over previous
"""Distributed multi-head attention kernel for 8 TRN2 NeuronCores.

Sharding: 8-way head parallel (2 heads per core), batches looped on-core.
Each core: QKV projection for its 2 heads over both batches, per-head
attention (softmax without max-subtraction — logits are small; denominators
come from a ones-column appended to V so they fall out of the attn@V
matmul), then per-head AllToAlls across all 8 cores exchange row-blocks
for head-blocks: block s = (batch s//4, rows-block s%4).  Core c ends up
with all 1024 inner dims for (batch c//4, rows [(c%4)*512, ...)) and runs
the full output projection + bias on that slice.

The attention middle is ACT-bound: 16.8M exps per core = ~145us of
Scalar-engine streaming, vs ~4.3us of PE work per 512-row block.  The
whole schedule is therefore built around keeping the exp stream dense:

- q/k are cast to fp8e4 post-projection so the QK^T matmuls run in
  DoubleRow perf mode (half the stream cycles; the DoubleRow k-tile
  pair dim is a stride-0 broadcast, compensated by halving the exp
  scale).  Costs ~0.9% relative error (1.3e-2 total, budget 2e-2),
  frees PE slack that the filler schedule spends.
- The first exp fires ~25us in: x^T row-block 0 fans out over 3 DMA
  queues, and the first ST block's jg groups interleave with the
  remaining k-projection chunks (group j only needs k block j//2).
- Every other projection (q/k rest, v of both batches) is chopped into
  ~1us filler units emitted between ST groups, sized so no unit delays
  the next ACT; q/k chunks split into two 4-ko halves with the PSUM
  accumulation held open across the gap.
- Phases run batch-major (h0b0, h1b0, h0b1, h1b1) so batch-1
  projections spread over two phases of filler slots; the head-0 A2A
  still hides under the last phase.
- A dummy AllGather at kernel start absorbs core boot skew so the real
  A2As (which share one serialized CC stream) fly at data-limited time.
- The head-1 receive interleaves the output projection: as core i's
  columns land in attnT, contraction step ko=i runs for output chunks
  0-5 (3 spare PSUM tiles = 6 open accumulations), overlapping most of
  the final matmuls with the receive chain.  Bias-adds ride the idle
  Scalar engine.  Junk "warm" matmuls cover the A2A-1 flight (HAM
  down-throttles the PE to 4/8 issue after idle gaps).

x and the weights are cast to bf16 on the host (bf16 is the compute
precision anyway) and x additionally arrives pre-transposed, so x^T
streams straight into SBUF with fully contiguous DMAs.  A2A-receive
transposes run on the PE array (XBAR transposes proved slow for narrow
blocks and corrupt data when issued on two HWDGE queues concurrently).

The per-core output is the TRANSPOSED final slice [1024, 512] (PSUM-major
writes stay contiguous); the host transposes during assembly.
"""
import numpy as np

import concourse.bass as bass
import concourse.mybir as mybir
from concourse import bacc
import concourse.tile as tile
from concourse.bass_utils import run_bass_kernel_spmd
from concourse.masks import make_identity

# problem constants (hardcoded; kernel.py must be self-contained)
B, N, DIM = 2, 2048, 1024
H, DH = 16, 64
INNER = H * DH            # 1024
SCALE = DIM ** -0.5       # 1/32  (module scales by dim**-0.5, not dim_head)
NCORES = 8
HPC = H // NCORES         # 2 heads per core
SH = HPC * DH             # 128 inner cols per core
ROWS = N // 4             # 512 output rows per core
P = 128
KO = DIM // P             # 8 contraction chunks
JC = N // P               # 16 row chunks
IB = 512                  # query block size
NIB = N // IB             # 4 query blocks
ISUB = IB // P            # 4
FP32 = mybir.dt.float32
BF16 = mybir.dt.bfloat16
F8 = mybir.dt.float8e4

REPLICA_GROUPS = [[0, 1, 2, 3, 4, 5, 6, 7]]

_NC_CACHE = {}

# set by the last kernel() call when BASS_KERNEL_TRACE=1 (for test.py)
LAST_RESULTS = None


def _build():
    nc = bacc.Bacc(num_devices=NCORES)

    x_ext = nc.declare_dram_parameter("x", [B * DIM, N], BF16, isOutput=False)
    wq_ext = nc.declare_dram_parameter("wq", [DIM, SH], BF16, isOutput=False)
    wk_ext = nc.declare_dram_parameter("wk", [DIM, SH], BF16, isOutput=False)
    wv_ext = nc.declare_dram_parameter("wv", [DIM, SH], BF16, isOutput=False)
    wo_ext = nc.declare_dram_parameter("wo", [DIM, DIM], BF16, isOutput=False)
    bo_ext = nc.declare_dram_parameter("bo", [DIM], FP32, isOutput=False)
    out_ext = nc.declare_dram_parameter("out", [DIM, ROWS], FP32, isOutput=True)

    with tile.TileContext(nc) as tc:
        with (
            tc.tile_pool(name="consts", bufs=1) as consts,
            tc.tile_pool(name="stage", bufs=3) as stage,
            tc.tile_pool(name="xt_pool", bufs=2) as xt_pool,
            tc.tile_pool(name="pt_pool", bufs=4) as pt_pool,
            tc.tile_pool(name="nrm", bufs=8) as nrm,
            tc.tile_pool(name="rst_pool", bufs=8) as rst_pool,
            tc.tile_pool(name="dram", bufs=1, space="DRAM") as dram,
            tc.tile_pool(name="st_psum", bufs=3, space="PSUM") as st_psum,
            tc.tile_pool(name="o_psum", bufs=2, space="PSUM") as o_psum,
        ):
            ident_bf = consts.tile([P, P], BF16)

            # persistent tensors
            wq_sb = consts.tile([P, KO, SH], BF16)
            wk_sb = consts.tile([P, KO, SH], BF16)
            wv_sb = consts.tile([P, KO, SH], BF16)
            wo_sb = consts.tile([P, KO, DIM], BF16)
            bias_sb = consts.tile([P, KO], FP32)
            # q/k kept in fp8e4 so the ST matmuls can use DoubleRow perf
            # mode (2 rows/cycle).  The DoubleRow "2 k-tiles" dim is a
            # stride-0 broadcast (both slots read the same data), which
            # doubles the logits; the exp ACT scale is halved to compensate.
            qT = consts.tile([P, B, HPC, N], F8)     # [d(+zero pad), b, h, i]
            kT = consts.tile([P, B, HPC, N], F8)
            v_aug = consts.tile([P, B, JC, HPC, DH + 1], BF16)
            out_rows = consts.tile([P, B, JC, SH], BF16)
            attnT = xt_pool.tile([P, KO, N], BF16, tag="xT", name="attnT")[
                :, :, :ROWS
            ]

            # tiny sync collective fired at kernel start: absorbs core
            # boot skew during the projection ramp so the real A2As fly
            # at their data-limited time instead of inheriting the skew
            # (the CC stream serializes collectives, so a skew-inflated
            # A2A-0 otherwise delays A2A-1's trigger).
            sync_in = dram.tile([P], BF16, name="sync_in")
            sync_out = dram.tile([NCORES, P], BF16, name="sync_out")
            a2a_in0 = dram.tile([NCORES, P, NIB, DH], BF16, name="a2a_in0")
            a2a_in1 = dram.tile([NCORES, P, NIB, DH], BF16, name="a2a_in1")
            a2a_out0 = dram.tile([NCORES, P, NIB, DH], BF16, name="a2a_out0")
            a2a_out1 = dram.tile([NCORES, P, NIB, DH], BF16, name="a2a_out1")
            a2a_ins = [a2a_in0, a2a_in1]
            a2a_outs = [a2a_out0, a2a_out1]

            # batch-0 q/k contraction pads zeroed on the Scalar engine
            # (idle until the first exp; keeps DVE free for the first q/k
            # casts and the gpsimd queue free for weight/xT DMAs);
            # batch-1 pads are emitted later, off the critical path.
            nc.scalar.memzero(qT[DH:P, 0, :, :])
            nc.scalar.memzero(kT[DH:P, 0, :, :])
            nc.vector.memset(v_aug[:, :, :, :, DH : DH + 1], 1.0)
            make_identity(nc, ident_bf)

            # junk operand for PE warm-up / warm-keeper matmuls (HAM needs
            # ~3.4us of sustained PE activity to un-throttle 1.2->2.4 GHz)
            junk = consts.tile([P, IB], BF16)
            nc.vector.memset(junk, 0.125)

            def pe_warm(n):
                for _ in range(n):
                    psw = st_psum.tile([P, 2, IB], FP32, tag="st", name="warm")
                    nc.tensor.matmul(
                        psw[:, 0, :], junk[:, 0:P], junk, start=True, stop=True
                    )

            def load_xT(b):
                """x[b]^T arrives pre-transposed from the host: straight
                contiguous loads spread over the DMA queues.  For batch 0
                the first row-block fans out over THREE queues (scalar is
                still idle pre-exp) so the k projection unblocks ~5us in."""
                xT = xt_pool.tile([P, KO, N], BF16, tag="xT", name="xT")
                for nb in range(NIB):
                    for ko in range(KO):
                        if b == 0 and nb == 0:
                            eng = [nc.sync, nc.gpsimd, nc.scalar][ko % 3]
                        else:
                            eng = [nc.sync, nc.gpsimd][ko % 2]
                        eng.dma_start(
                            xT[:, ko, nb * IB : (nb + 1) * IB],
                            x_ext[
                                b * DIM + ko * P : b * DIM + (ko + 1) * P,
                                nb * IB : (nb + 1) * IB,
                            ],
                        )
                return xT

            qk_open = {}

            def qk_half(b, xT, w_sb, dstT, nb, half):
                """One half (4 ko steps) of a q/k projection chunk; the
                PSUM accumulation stays open between halves so a half fits
                a single filler slot without overrunning the ACT cadence."""
                key = (id(dstT), b, nb)
                if half == 0:
                    ps2 = st_psum.tile([P, 2, IB], FP32, tag="st", name="qk_ps")
                    qk_open[key] = ps2
                ps = qk_open[key][:, 0, :]
                for ko in range(half * 4, half * 4 + 4):
                    nc.tensor.matmul(
                        ps,
                        w_sb[:, ko, :],
                        xT[:, ko, nb * IB : (nb + 1) * IB],
                        start=(ko == 0),
                        stop=(ko == KO - 1),
                    )
                if half == 1:
                    del qk_open[key]
                    for h in range(HPC):
                        nc.vector.tensor_copy(
                            dstT[0:DH, b, h, nb * IB : (nb + 1) * IB],
                            ps[h * DH : (h + 1) * DH, :],
                        )

            def qk_chunk(b, xT, w_sb, dstT, nbs):
                for nb in nbs:
                    qk_half(b, xT, w_sb, dstT, nb, 0)
                    qk_half(b, xT, w_sb, dstT, nb, 1)

            def qk_proj(b, xT):
                # k first (attention's dots consume kT earliest), then q
                qk_chunk(b, xT, wk_sb, kT, range(NIB))
                qk_chunk(b, xT, wq_sb, qT, range(NIB))

            def load_qkv_weights():
                for w_ext, w_sb in (
                    (wk_ext, wk_sb),
                    (wq_ext, wq_sb),
                    (wv_ext, wv_sb),
                ):
                    nc.gpsimd.dma_start(
                        w_sb, w_ext.rearrange("(ko kp) c -> kp ko c", kp=P)
                    )

            def load_out_weights():
                nc.gpsimd.dma_start(
                    wo_sb, wo_ext.rearrange("(ko kp) c -> kp ko c", kp=P)
                )
                nc.gpsimd.dma_start(
                    bias_sb, bo_ext.rearrange("(co cp) -> cp co", cp=P)
                )

            def v_proj(b, xT, mts=None):
                for mt in (range(JC) if mts is None else mts):
                    psv2 = st_psum.tile([P, 2, IB], FP32, tag="st", name="v_ps")
                    ps_v = psv2[:, 0, :SH]
                    for ko in range(KO):
                        nc.tensor.matmul(
                            ps_v,
                            xT[:, ko, mt * P : (mt + 1) * P],
                            wv_sb[:, ko, :],
                            start=(ko == 0),
                            stop=(ko == KO - 1),
                        )
                    nc.vector.tensor_copy(
                        v_aug[:, b, mt, :, 0:DH],
                        ps_v.rearrange("p (h d) -> p h d", d=DH),
                    )

            ptiles = {}

            def st_exp(h, b, ib, fillers=None):
                """ST = k@q.T per j-chunk (fp8 DoubleRow), exp on ACT.
                fillers: up to 8 closures of ~1-2us of PE work, one
                emitted before each jg group, so projection work streams
                through ACT-bound rounds without ever parking more than
                one filler unit ahead of the next ST group."""
                ptile = pt_pool.tile([P, JC, IB], BF16, tag="pt", name="ptile")
                ptiles[(h, b, ib)] = ptile
                for jg in range(JC // 2):
                    if fillers is not None and jg < len(fillers):
                        f = fillers[jg]
                        if f is not None:
                            f()
                    ps_st = st_psum.tile(
                        [P, 2, IB], FP32, tag="st", name="st_ps"
                    )
                    for u in range(2):
                        jc = jg * 2 + u
                        nc.tensor.matmul(
                            ps_st[:, u, :],
                            kT[:, None, b, h, jc * P : (jc + 1) * P]
                            .broadcast_to([P, 2, P]),
                            qT[:, None, b, h, ib * IB : (ib + 1) * IB]
                            .broadcast_to([P, 2, IB]),
                            start=True,
                            stop=True,
                            perf_mode=mybir.MatmulPerfMode.DoubleRow,
                        )
                    nc.scalar.activation(
                        ptile[:, jg * 2 : (jg + 1) * 2, :],
                        ps_st,
                        mybir.ActivationFunctionType.Exp,
                        scale=SCALE / 2,
                    )

            def av(h, b, ib, weave=False):
                """attn@V with the denominator in column DH.  All 4 i-sub
                accumulations of a block share one PSUM bank so the 'po'
                slots rotate once per block (normalizes emitted after all
                64 matmuls so the bank tracker doesn't interleave PE
                writes with DVE reads)."""
                po = h * DH
                ptile = ptiles.pop((h, b, ib))
                ps_o4 = o_psum.tile(
                    [P, ISUB, DH + 1], FP32, tag="po", name="o_ps"
                )
                if weave:
                    # jc-outer: each jc pair only needs its own exp group,
                    # so 56 of the 64 matmuls run while later exps stream;
                    # only the last 8 trail the final ACT of the block.
                    # Warm units hold HAM up through the narrow matmuls.
                    for jc in range(JC):
                        if jc % 4 == 0:
                            pe_warm(1)
                        for isub in range(ISUB):
                            # single start: the pending-zero region is the
                            # whole 2KB bank, so the first write of each
                            # interleaved chain still reads as zeroed
                            nc.tensor.matmul(
                                ps_o4[:, isub, :],
                                ptile[:, jc, isub * P : (isub + 1) * P],
                                v_aug[:, b, jc, h, :],
                                start=(jc == 0 and isub == 0),
                                stop=(jc == JC - 1),
                            )
                else:
                    for isub in range(ISUB):
                        for jc in range(JC):
                            nc.tensor.matmul(
                                ps_o4[:, isub, :],
                                ptile[:, jc, isub * P : (isub + 1) * P],
                                v_aug[:, b, jc, h, :],
                                start=(jc == 0),
                                stop=(jc == JC - 1),
                            )
                for isub in range(ISUB):
                    ic = ib * ISUB + isub
                    recip = nrm.tile([P, 1], FP32, tag="recip", name="recip")
                    nc.vector.reciprocal(recip, ps_o4[:, isub, DH : DH + 1])
                    nc.vector.tensor_scalar_mul(
                        out_rows[:, b, ic, po : po + DH],
                        ps_o4[:, isub, 0:DH],
                        recip,
                    )
                # block (h, b, ib) complete -> stage its A2A input
                s = b * NIB + ib
                nc.sync.dma_start(
                    a2a_ins[h][s],
                    out_rows[
                        :, b, ib * ISUB : (ib + 1) * ISUB, po : po + DH
                    ],
                )

            def a2a_exchange(h):
                nc.gpsimd.collective_compute(
                    "AllToAll",
                    mybir.AluOpType.bypass,
                    replica_groups=REPLICA_GROUPS,
                    ins=[a2a_ins[h].opt()],
                    outs=[a2a_outs[h].opt()],
                )

            def receive(h, eng=None):
                po = h * DH
                eng = eng or nc.sync
                rstages = []
                for i in range(NCORES):
                    rstage = rst_pool.tile(
                        [P, NIB, DH], BF16, tag="rstage", name="rstage"
                    )
                    eng.dma_start(rstage, a2a_outs[h][i])
                    rstages.append(rstage)
                for i in range(NCORES):
                    rps = st_psum.tile([DH, NIB, P], BF16, tag="st", name="r_ps")
                    for q in range(NIB):
                        nc.tensor.transpose(
                            rps[:, q, :], rstages[i][:, q, :], ident_bf
                        )
                    nc.vector.tensor_copy(attnT[po : po + DH, i, :], rps)

            def emit_out(ps_f, cc):
                # bias-add on the Scalar engine (idle after the last exp;
                # keeps DVE free for the receive copies in the tail)
                of = stage.tile([P, ROWS], FP32, tag="of", name="of")
                nc.scalar.activation(
                    of,
                    ps_f,
                    mybir.ActivationFunctionType.Identity,
                    bias=bias_sb[:, cc : cc + 1],
                )
                nc.sync.dma_start(out_ext[cc * P : (cc + 1) * P, :], of)

            def receive1_final():
                """Head-1 receive with the output projection interleaved:
                as each source core's columns land in attnT, the matching
                ko contraction step runs for output chunks 0-5 (3 spare
                PSUM st tiles = 6 open accumulations; chunks 6-7 run
                monolithically at the end).  Overlaps ~13us of final
                matmuls with the receive chain instead of serializing."""
                po = DH
                rstages = []
                for i in range(NCORES):
                    rstage = rst_pool.tile(
                        [P, NIB, DH], BF16, tag="rstage", name="rstage"
                    )
                    (nc.sync if i % 2 == 0 else nc.scalar).dma_start(
                        rstage, a2a_outs[1][i]
                    )
                    rstages.append(rstage)
                fps = [
                    st_psum.tile([P, 2, IB], FP32, tag="st", name="f_ps")
                    for _ in range(3)
                ]
                for i in range(NCORES):
                    rps = o_psum.tile([DH, NIB, P], BF16, tag="po", name="r_ps")
                    for q in range(NIB):
                        nc.tensor.transpose(
                            rps[:, q, :], rstages[i][:, q, :], ident_bf
                        )
                    nc.vector.tensor_copy(attnT[po : po + DH, i, :], rps)
                    for cc in range(6):
                        nc.tensor.matmul(
                            fps[cc // 2][:, cc % 2, :ROWS],
                            wo_sb[:, i, cc * P : (cc + 1) * P],
                            attnT[:, i, :],
                            start=(i == 0),
                            stop=(i == KO - 1),
                        )
                for cc in range(6):
                    emit_out(fps[cc // 2][:, cc % 2, :ROWS], cc)
                for cc in range(6, KO):
                    psf2 = st_psum.tile([P, 2, IB], FP32, tag="st", name="f_ps")
                    ps_f = psf2[:, 0, :ROWS]
                    for ko in range(KO):
                        nc.tensor.matmul(
                            ps_f,
                            wo_sb[:, ko, cc * P : (cc + 1) * P],
                            attnT[:, ko, :],
                            start=(ko == 0),
                            stop=(ko == KO - 1),
                        )
                    emit_out(ps_f, cc)

            # ---- program order: ACT-dense schedule.  The attention phase
            # is ACT-bound (~9us of exp per block vs ~4.3us of PE work with
            # fp8 STs), so the first exp is pulled as early as possible
            # (k(0) + q(0) block 0 + ST block 0 only) and every remaining
            # projection is emitted as PE filler between ST groups, sized
            # so each ST group lands before the ACT queue drains.  Phases
            # run batch-major (h0b0, h1b0, h0b1, h1b1) so batch-1
            # projections can spread across TWO phases of filler; the
            # head-0 A2A still fully hides under phase h1b1. ----
            kh = lambda b, xT, nb, hf: (
                lambda: qk_half(b, xT, wk_sb, kT, nb, hf)
            )
            qh = lambda b, xT, nb, hf: (
                lambda: qk_half(b, xT, wq_sb, qT, nb, hf)
            )
            qc = lambda b, xT, nb: (lambda: qk_chunk(b, xT, wq_sb, qT, [nb]))
            vc = lambda b, xT, m0, m1: (lambda: v_proj(b, xT, range(m0, m1)))
            wu = lambda: pe_warm(1)

            load_qkv_weights()
            xT0 = load_xT(0)
            nc.gpsimd.collective_compute(
                "AllGather",
                mybir.AluOpType.bypass,
                replica_groups=REPLICA_GROUPS,
                ins=[sync_in.opt()],
                outs=[sync_out.opt()],
            )
            pe_warm(2)
            qk_chunk(0, xT0, wk_sb, kT, [0])
            qk_chunk(0, xT0, wq_sb, qT, [0])
            # first ST block interleaves with the remaining k chunks (its
            # jg group j only needs the k block j//2 complete), so exp
            # starts right after the first k/q blocks land.
            st_exp(0, 0, 0, fillers=[
                kh(0, xT0, 1, 0), kh(0, xT0, 1, 1),
                kh(0, xT0, 2, 0), kh(0, xT0, 2, 1),
                kh(0, xT0, 3, 0), kh(0, xT0, 3, 1),
                qh(0, xT0, 1, 0), qh(0, xT0, 1, 1),
            ])
            # batch-1 q/k pads, off the startup critical path
            nc.gpsimd.memset(qT[DH:P, 1, :, :], 0.0)
            nc.gpsimd.memset(kT[DH:P, 1, :, :], 0.0)
            st_exp(0, 0, 1, fillers=[
                qh(0, xT0, 2, 0), qh(0, xT0, 2, 1),
                vc(0, xT0, 0, 1), vc(0, xT0, 1, 2), vc(0, xT0, 2, 3),
                vc(0, xT0, 3, 4), vc(0, xT0, 4, 5), vc(0, xT0, 5, 6),
            ])
            xT1 = load_xT(1)
            load_out_weights()
            st_exp(0, 0, 2, fillers=[
                qh(0, xT0, 3, 0), qh(0, xT0, 3, 1),
                vc(0, xT0, 6, 7), vc(0, xT0, 7, 8), vc(0, xT0, 8, 9),
                vc(0, xT0, 9, 10), vc(0, xT0, 10, 11), vc(0, xT0, 11, 12),
            ])
            st_exp(0, 0, 3, fillers=[
                vc(0, xT0, 12, 13), vc(0, xT0, 13, 14), vc(0, xT0, 14, 15),
                vc(0, xT0, 15, 16), vc(1, xT1, 0, 1), vc(1, xT1, 1, 2),
                vc(1, xT1, 2, 3), vc(1, xT1, 3, 4),
            ])
            # phase h1b0: fillers = batch-1 k chunks (halves in adjacent
            # slots so the open PSUM chain spans only one ST group) + v
            av(0, 0, 0)
            st_exp(1, 0, 0, fillers=[
                kh(1, xT1, 0, 0), kh(1, xT1, 0, 1),
                vc(1, xT1, 4, 5), None, vc(1, xT1, 5, 6), None,
            ])
            av(0, 0, 1)
            st_exp(1, 0, 1, fillers=[
                kh(1, xT1, 1, 0), kh(1, xT1, 1, 1),
                vc(1, xT1, 6, 7), None, vc(1, xT1, 7, 8), None,
            ])
            av(0, 0, 2)
            st_exp(1, 0, 2, fillers=[
                kh(1, xT1, 2, 0), kh(1, xT1, 2, 1),
                vc(1, xT1, 8, 9), None, vc(1, xT1, 9, 10), None,
            ])
            av(0, 0, 3)
            st_exp(1, 0, 3, fillers=[
                kh(1, xT1, 3, 0), kh(1, xT1, 3, 1),
                vc(1, xT1, 10, 11), None, vc(1, xT1, 11, 12), None,
            ])
            # phase h0b1: fillers = batch-1 q chunks, just in time (the
            # slot-0 q chunk feeds this very block's STs)
            av(1, 0, 0)
            st_exp(0, 1, 0, fillers=[
                qc(1, xT1, 0), vc(1, xT1, 12, 13), None,
                vc(1, xT1, 13, 14), None, None,
            ])
            av(1, 0, 1)
            st_exp(0, 1, 1, fillers=[
                qc(1, xT1, 1), vc(1, xT1, 14, 15), None,
                vc(1, xT1, 15, 16), None, None,
            ])
            av(1, 0, 2)
            st_exp(0, 1, 2, fillers=[
                qc(1, xT1, 2), None, None, None, None
            ])
            av(1, 0, 3)
            st_exp(0, 1, 3, fillers=[
                qc(1, xT1, 3), None, None, None, None
            ])
            # phase h1b1; head-0 finishes at av(0,1,3) -> exchange(0)
            # flies under the tail of this phase.  Small warm fillers keep
            # the PE above HAM's low-activity threshold now that all real
            # projection work is done.
            av(0, 1, 0)
            st_exp(1, 1, 0, fillers=[wu, None, None, None, wu])
            av(0, 1, 1)
            st_exp(1, 1, 1, fillers=[wu, None, None, None, wu])
            av(0, 1, 2)
            st_exp(1, 1, 2, fillers=[wu, None, None, None, wu])
            av(0, 1, 3)
            a2a_exchange(0)
            st_exp(1, 1, 3, fillers=[wu, None, None, None, wu])
            av(1, 1, 0, weave=True)
            av(1, 1, 1, weave=True)
            av(1, 1, 2, weave=True)
            av(1, 1, 3, weave=True)
            a2a_exchange(1)
            receive(0, nc.scalar)  # coll-0 landed by now; ACT queue free
            pe_warm(40)            # PE activity across the A2A-1 flight
            receive1_final()

    nc.finalize()
    return nc


def _get_nc():
    if "nc" not in _NC_CACHE:
        _NC_CACHE["nc"] = _build()
    return _NC_CACHE["nc"]


def kernel(**inputs) -> np.ndarray:
    import os

    import ml_dtypes

    global LAST_RESULTS

    bf16 = ml_dtypes.bfloat16
    x = np.asarray(inputs["x"], dtype=np.float32)
    W_qkv = np.asarray(inputs["W_qkv"], dtype=np.float32)
    W_out = np.asarray(inputs["W_out"], dtype=np.float32)
    b_out = np.ascontiguousarray(np.asarray(inputs["b_out"], dtype=np.float32))

    x_bf = np.ascontiguousarray(
        x.transpose(0, 2, 1).reshape(B * DIM, N).astype(bf16)
    )
    wo_bf = np.ascontiguousarray(W_out.astype(bf16))
    wqkv_bf = W_qkv.astype(bf16)

    nc = _get_nc()

    in_maps = []
    for c in range(NCORES):
        in_maps.append(
            {
                "x": x_bf,
                "wq": np.ascontiguousarray(
                    wqkv_bf[:, 0 * INNER + c * SH : 0 * INNER + (c + 1) * SH]
                ),
                "wk": np.ascontiguousarray(
                    wqkv_bf[:, 1 * INNER + c * SH : 1 * INNER + (c + 1) * SH]
                ),
                "wv": np.ascontiguousarray(
                    wqkv_bf[:, 2 * INNER + c * SH : 2 * INNER + (c + 1) * SH]
                ),
                "wo": wo_bf,
                "bo": b_out,
            }
        )

    trace = os.environ.get("BASS_KERNEL_TRACE", "0") == "1"
    res = run_bass_kernel_spmd(
        nc, in_maps, core_ids=list(range(NCORES)), trace=trace
    )
    LAST_RESULTS = res

    y = np.empty((B, N, DIM), dtype=np.float32)
    for c in range(NCORES):
        b, r = c // 4, c % 4
        y[b, r * ROWS : (r + 1) * ROWS, :] = res.results[c]["out"].T
    return y

